# revision 1
# baseline (speedup 1.0000x reference)
"""BiMamba (bimamba_type='v2') Trainium2 Bass kernel.

Data-parallel over the fused B*N=828 (padded to 832) sequence axis across 8
NeuronCores (104 sequences/core). Per-core device program:
  - channels d (d_inner=256) -> 2 partition tiles of 128
  - scan-phase tensors laid out [p=d-tile, (branch, n_state, batch, time)],
    time innermost & contiguous; the selective scan runs as one DVE
    tensor_tensor_scan per (d-tile, chunk); dA is zeroed at t=0 so the
    recurrence resets at every (branch, n, sequence) segment boundary.
  - softplus is unavailable in the ACT tables -> dt = ln(1 + exp(x));
    rstd = exp(-0.5*ln(var+eps)) keeps ACT work in {exp, ln} + silu.
Engines: PE matmuls + LN partition-sums + stat broadcasts; ACT exp/ln/silu;
DVE elementwise + scan (bf16 operands where the 2x mode applies); DMA
broadcasts via DRAM round-trip.
"""

import numpy as np

import concourse.bass as bass
import concourse.tile as tile
from concourse import bacc, mybir
from concourse.bass_utils import run_bass_kernel_spmd

F32 = mybir.dt.float32
BF16 = mybir.dt.bfloat16
AF = mybir.ActivationFunctionType
ALU = mybir.AluOpType

B, T, N, C = 4, 24, 207, 128
DI = 256
DS = 16
RK = 8
EPS = 1e-5
NCORES = 8
BSEQ = 832                   # padded B*N
BC = BSEQ // NCORES          # 104 sequences per core
NCHUNK = 8
CB = BC // NCHUNK            # 13 seqs per chunk
CBT = CB * T                 # 312 tokens per chunk


def _pbcast(ap, parts=128):
    """DRAM-source AP for a DMA that replicates data across `parts`
    partitions (prepends a 0-stride partition dim)."""
    a = [[0, parts]] + [list(x) for x in ap.ap]
    return bass.AP(tensor=ap.tensor, offset=ap.offset, ap=a)


def _rev_t(ap):
    """Reverse the last free dim of an AP."""
    a = [list(x) for x in ap.ap]
    st, ct = a[-1]
    off = ap.offset + st * (ct - 1)
    a[-1] = [-st, ct]
    return bass.AP(tensor=ap.tensor, offset=off, ap=a)


def _zstride(ap, dim, count):
    """Insert a 0-stride free dim at position `dim` (0 = just after the
    partition dim)."""
    a = [list(x) for x in ap.ap]
    a.insert(1 + dim, [0, count])
    return bass.AP(tensor=ap.tensor, offset=ap.offset, ap=a)


def build_program(a_pow):
    """a_pow: 16 floats = A[0, :] (decay coeffs, d-independent, both
    branches identical; asserted on host)."""
    nc = bacc.Bacc("TRN2", target_bir_lowering=False, debug=False,
                   enable_asserts=False, num_devices=NCORES)

    def din(name, shape):
        return nc.dram_tensor(name, shape, F32, kind="ExternalInput").ap()

    xin = din("xin", [C, BC, T])
    w_in = din("w_in", [C, 4 * C])
    convw = din("convw", [128, 2, 2, 4])
    convb = din("convb", [128, 2, 2, 1])
    xw = din("xw", [128, 2, 2, 40])
    dtw = din("dtw", [RK, 2, DI])
    dtb = din("dtb", [128, 2, 2, 1])
    dpc = din("dpc", [128, 2, 2, 1])
    wout = din("wout", [128, 2, C])
    ln1g = din("ln1g", [C, 1])
    ln1b = din("ln1b", [C, 1])
    ln2g = din("ln2g", [C, 1])
    ln2b = din("ln2b", [C, 1])
    out = nc.dram_tensor("out", [C, BC, T], F32, kind="ExternalOutput").ap()

    with tile.TileContext(nc) as tc, \
         tc.tile_pool(name="weights", bufs=1) as wp, \
         tc.tile_pool(name="small", bufs=2) as sp, \
         tc.tile_pool(name="stats", bufs=2) as stp, \
         tc.tile_pool(name="big", bufs=2) as bp, \
         tc.tile_pool(name="bcrep", bufs=1) as bcp, \
         tc.tile_pool(name="dram", bufs=2, space="DRAM") as drp, \
         tc.tile_pool(name="psA", bufs=2, space="PSUM") as psA, \
         tc.tile_pool(name="psB", bufs=1, space="PSUM") as psB, \
         tc.tile_pool(name="psC", bufs=2, space="PSUM") as psC:

        def load_w(name, ap_src, shape, dt=F32):
            t = wp.tile(shape, dt, tag=name, name=name)
            nc.sync.dma_start(t[:], ap_src)
            return t

        w_in_sb = load_w("w_in", w_in, [C, 4 * C])
        convw_sb = load_w("convw", convw, [128, 2, 2, 4])
        convb_sb = load_w("convb", convb, [128, 2, 2, 1])
        xw_sb = load_w("xw", xw, [128, 2, 2, 40])
        dtw_sb = load_w("dtw", dtw, [RK, 2, DI])
        dtb_sb = load_w("dtb", dtb, [128, 2, 2, 1])
        dpc_sb = load_w("dpc", dpc, [128, 2, 2, 1])
        wout_sb = load_w("wout", wout, [128, 2, C])
        ln1g_sb = load_w("ln1g", ln1g, [C, 1])
        ln1b_sb = load_w("ln1b", ln1b, [C, 1])
        ln2g_sb = load_w("ln2g", ln2g, [C, 1])
        ln2b_sb = load_w("ln2b", ln2b, [C, 1])
        ones_sb = wp.tile([C, 1], F32, tag="ones")
        nc.vector.memset(ones_sb[:], 1.0)
        eps_sb = wp.tile([C, 1], F32, tag="eps")
        nc.vector.memset(eps_sb[:], EPS)
        ones_row = wp.tile([1, C], F32, tag="ones_row")
        nc.vector.memset(ones_row[:], 1.0)

        def layernorm(src_f32, g_sb, b_sb, dst):
            """LN over the partition (channel) dim of src [C, CBT] -> dst."""
            sq = sp.tile([C, CBT], F32, tag="ln_sq")
            nc.vector.tensor_mul(sq[:], src_f32, src_f32)
            ps_s = psC.tile([1, CBT], F32, tag="ps_stat", name="ps_s")
            ps_q = psC.tile([1, CBT], F32, tag="ps_stat", name="ps_q")
            nc.tensor.matmul(ps_s[:], ones_sb[:], src_f32, start=True, stop=True)
            nc.tensor.matmul(ps_q[:], ones_sb[:], sq[:], start=True, stop=True)
            mean = stp.tile([1, CBT], F32, tag="mean")
            nc.vector.tensor_scalar(mean[:], ps_s[:], 1.0 / C, None, ALU.mult)
            var = stp.tile([1, CBT], F32, tag="var")
            nc.vector.tensor_scalar(var[:], ps_q[:], 1.0 / C, None, ALU.mult)
            m2 = stp.tile([1, CBT], F32, tag="m2")
            nc.vector.tensor_mul(m2[:], mean[:], mean[:])
            nc.vector.tensor_sub(var[:], var[:], m2[:])
            # rstd = (var+eps)^-0.5 = exp(-0.5*ln(var+eps)); no sqrt table
            nc.scalar.activation(var[:], var[:], AF.Ln, bias=eps_sb[0:1, 0:1])
            nc.scalar.activation(var[:], var[:], AF.Exp, scale=-0.5)
            mean_r = psC.tile([C, CBT], F32, tag="ps_stat", name="mean_r")
            nc.tensor.matmul(mean_r[:], ones_row[:], mean[:],
                             start=True, stop=True)
            rstd_r = psC.tile([C, CBT], F32, tag="ps_stat", name="rstd_r")
            nc.tensor.matmul(rstd_r[:], ones_row[:], var[:],
                             start=True, stop=True)
            nc.vector.tensor_sub(dst, src_f32, mean_r[:])
            nc.vector.tensor_mul(dst, dst, rstd_r[:])
            nc.vector.tensor_scalar(dst, dst, g_sb[:, 0:1], b_sb[:, 0:1],
                                    ALU.mult, ALU.add)

        for ch in range(NCHUNK):
            b0 = ch * CB
            u = sp.tile([C, CB, T], F32, tag="u")
            nc.sync.dma_start(u[:], xin[:, b0:b0 + CB, :])
            uf = u[:].rearrange("p b t -> p (b t)")

            hln = sp.tile([C, CBT], F32, tag="hln")
            layernorm(uf, ln1g_sb, ln1b_sb, hln[:])

            # in_proj (m-tiles: xx0 xx1 z0 z1)
            xx = [sp.tile([128, CB, T], F32, tag=f"xx{ti}", name=f"xx{ti}")
                  for ti in range(2)]
            sz = [sp.tile([128, CB, T], BF16, tag=f"sz{ti}", name=f"sz{ti}")
                  for ti in range(2)]
            for mt in range(4):
                ps_xz = psA.tile([128, CBT], F32, tag="ps_xz")
                nc.tensor.matmul(ps_xz[:], w_in_sb[:, mt * 128:(mt + 1) * 128],
                                 hln[:], start=True, stop=True)
                dst = xx[mt][:] if mt < 2 else sz[mt - 2][:]
                pv = ps_xz[:].rearrange("p (b t) -> p b t", t=T)
                if mt < 2:
                    nc.vector.tensor_copy(dst, pv)
                else:
                    nc.scalar.activation(dst, pv, AF.Silu)

            # causal depthwise conv + silu; xc2[ti]: [128,(br,b,t)]
            xc2 = [sp.tile([128, 2, CB, T], F32, tag=f"xc{ti}", name=f"xc{ti}", bufs=1)
                   for ti in range(2)]
            for ti in range(2):
                xxv = xx[ti][:]
                for br in range(2):
                    acc = xc2[ti][:, br, :, :]
                    w3 = convw_sb[:, br, ti, 3:4]
                    src3 = xxv if br == 0 else _rev_t(xxv)
                    nc.vector.tensor_scalar(acc, src3, w3, None, ALU.mult)
                    for k in range(3):
                        src = xxv[:, :, :T - (3 - k)] if br == 0 \
                            else _rev_t(xxv[:, :, 3 - k:])
                        nc.vector.scalar_tensor_tensor(
                            acc[:, :, 3 - k:], src,
                            convw_sb[:, br, ti, k:k + 1],
                            acc[:, :, 3 - k:], ALU.mult, ALU.add)
                    nc.scalar.activation(acc, acc, AF.Silu,
                                         bias=convb_sb[:, br, ti, 0:1])

            # xproj -> x_dbl [40, CBT] per branch; stage dtraw f32, B/C bf16
            dtraw, bc_bf = [None, None], [None, None]
            for br in range(2):
                ps_xd = psB.tile([40, CBT], F32, tag="ps_xd")
                for ti in range(2):
                    nc.tensor.matmul(ps_xd[:], xw_sb[:, br, ti, :],
                                     xc2[ti][:, br, :, :].rearrange(
                                         "p b t -> p (b t)"),
                                     start=(ti == 0), stop=(ti == 1))
                bc_bf[br] = stp.tile([32, CBT], BF16, tag=f"bcbf{br}",
                                     name=f"bcbf{br}")
                nc.vector.tensor_copy(bc_bf[br][:], ps_xd[0:32, :])
                dtraw[br] = stp.tile([RK, CBT], F32, tag=f"dtraw{br}",
                                     name=f"dtraw{br}")
                nc.vector.tensor_copy(dtraw[br][:], ps_xd[32:40, :])

            # B_rep / C_rep via DRAM round-trip broadcast
            b1d = drp.tile([2, DS, CB, T], BF16, tag="b1d")
            c1d = drp.tile([2, DS, CB, T], BF16, tag="c1d")
            for br in range(2):
                nc.sync.dma_start(b1d[br, :, :, :],
                                  bc_bf[br][0:DS, :].rearrange(
                                      "p (b t) -> p b t", t=T))
                nc.sync.dma_start(c1d[br, :, :, :],
                                  bc_bf[br][DS:32, :].rearrange(
                                      "p (b t) -> p b t", t=T))
                # (bc_bf rows: 0:16 = B, 16:32 = C; dtraw from rows 32:40)
            brep = bcp.tile([128, 2 * DS * CBT], BF16, tag="brep")
            nc.sync.dma_start(
                brep[:], _pbcast(b1d[:].rearrange("a n b t -> (a n b t)")))
            crep = bcp.tile([128, 2 * DS * CBT], BF16, tag="crep")
            nc.sync.dma_start(
                crep[:], _pbcast(c1d[:].rearrange("a n b t -> (a n b t)")))
            brep5 = brep[:].rearrange("p (a n b t) -> p a n b t",
                                      a=2, n=DS, b=CB)
            crep5 = crep[:].rearrange("p (a n b t) -> p a n b t",
                                      a=2, n=DS, b=CB)

            # dtproj; dt = ln(1 + exp(x + bias))  (softplus table is absent)
            dt2 = [sp.tile([128, 2, CB, T], F32, tag=f"dt{ti}", name=f"dt{ti}", bufs=1)
                   for ti in range(2)]
            for br in range(2):
                for ti in range(2):
                    ps_dt = psB.tile([128, CBT], F32, tag="ps_dt")
                    nc.tensor.matmul(ps_dt[:],
                                     dtw_sb[:, br, ti * 128:(ti + 1) * 128],
                                     dtraw[br][:], start=True, stop=True)
                    slab = dt2[ti][:, br, :, :]
                    nc.scalar.activation(
                        slab, ps_dt[:].rearrange("p (b t) -> p b t", t=T),
                        AF.Exp, bias=dtb_sb[:, br, ti, 0:1])
                    nc.scalar.activation(slab, slab, AF.Ln, bias=1.0)

            ps_o = psB.tile([128, CBT], F32, tag="ps_o")
            for ti in range(2):
                # du = dt * xc  (bf16)
                du2 = bp.tile([128, 2, CB, T], BF16, tag="du2")
                nc.vector.tensor_mul(du2[:], dt2[ti][:], xc2[ti][:])

                # dA[n] = exp(a_n * dt); zero at t=0 (scan segment reset)
                dA = bp.tile([128, 2, DS, CB, T], BF16, tag="dA")
                for n in range(DS):
                    nc.scalar.activation(dA[:, :, n, :, :], dt2[ti][:],
                                         AF.Exp, scale=float(a_pow[n]))
                nc.gpsimd.memset(dA[:, :, :, :, 0:1], 0.0)

                # dBu = du (bcast over n) * B_rep
                dBu = bp.tile([128, 2, DS, CB, T], BF16, tag="dBu")
                nc.vector.tensor_mul(dBu[:], _zstride(du2[:], 1, DS), brep5)

                # selective scan along (br, n, b, t) flattened free axis
                h = bp.tile([128, 2, DS, CB, T], BF16, tag="h", bufs=1)
                nc.vector.tensor_tensor_scan(
                    h[:].rearrange("p a n b t -> p (a n b t)"),
                    dA[:].rearrange("p a n b t -> p (a n b t)"),
                    dBu[:].rearrange("p a n b t -> p (a n b t)"),
                    0.0, ALU.mult, ALU.add)

                # hc = h * C_rep; tree-reduce over n into the n=0 slab
                nc.vector.tensor_mul(h[:], h[:], crep5)
                for w in (8, 4, 2, 1):
                    nc.vector.tensor_add(h[:, :, 0:w, :, :],
                                         h[:, :, 0:w, :, :],
                                         h[:, :, w:2 * w, :, :])

                # per-branch y_br = scan_out + Dp_br*xc_br (branch coords),
                # then y = y_f + rev(y_b), gated by silu(z)
                yf = sp.tile([128, CB, T], F32, tag=f"yf{ti}", name=f"yf{ti}")
                nc.vector.scalar_tensor_tensor(
                    yf[:], xc2[ti][:, 0, :, :], dpc_sb[:, 0, ti, 0:1],
                    h[:, 0, 0, :, :], ALU.mult, ALU.add)
                yb = sp.tile([128, CB, T], F32, tag=f"yb{ti}", name=f"yb{ti}")
                nc.vector.scalar_tensor_tensor(
                    yb[:], xc2[ti][:, 1, :, :], dpc_sb[:, 1, ti, 0:1],
                    h[:, 1, 0, :, :], ALU.mult, ALU.add)
                ypre = sp.tile([128, CB, T], F32, tag=f"ypre{ti}",
                               name=f"ypre{ti}")
                nc.vector.tensor_add(ypre[:], yf[:], _rev_t(yb[:]))
                nc.vector.tensor_mul(ypre[:], ypre[:], sz[ti][:])

                nc.tensor.matmul(ps_o[:], wout_sb[:, ti, :],
                                 ypre[:].rearrange("p b t -> p (b t)"),
                                 start=(ti == 0), stop=(ti == 1))

            # LN2 + residual
            o_sb = sp.tile([C, CBT], F32, tag="o_sb")
            nc.vector.tensor_copy(o_sb[:], ps_o[:])
            o_ln = sp.tile([C, CBT], F32, tag="o_ln")
            layernorm(o_sb[:], ln2g_sb, ln2b_sb, o_ln[:])
            nc.vector.tensor_add(o_ln[:], o_ln[:], uf)
            nc.sync.dma_start(out[:, b0:b0 + CB, :],
                              o_ln[:].rearrange("p (b t) -> p b t", t=T))

    nc.finalize()
    return nc


def _prep(inputs):
    f = lambda k: np.ascontiguousarray(np.asarray(inputs[k], np.float32))
    x = f("x")
    u_all = x.transpose(0, 2, 1, 3).reshape(B * N, T, C)
    u_pad = np.zeros((BSEQ, T, C), np.float32)
    u_pad[:B * N] = u_all
    xin = [np.ascontiguousarray(u_pad[i * BC:(i + 1) * BC].transpose(2, 0, 1))
           for i in range(NCORES)]

    A = -np.exp(f("A_log"))
    Ab = -np.exp(f("A_b_log"))
    assert np.allclose(A, A[0:1], rtol=1e-5), "A must be d-independent"
    assert np.allclose(Ab, A, rtol=1e-5), "A_b must equal A"
    a_pow = [float(v) for v in A[0]]

    cw = np.stack([f("conv_w")[:, 0, :], f("conv_w_b")[:, 0, :]])   # [2,256,4]
    cb = np.stack([f("conv_b"), f("conv_b_b")])[..., None]          # [2,256,1]
    xw_ro = np.concatenate([f("xproj_w")[RK:], f("xproj_w")[:RK]])
    xw_ro_b = np.concatenate([f("xproj_w_b")[RK:], f("xproj_w_b")[:RK]])
    xwm = np.stack([xw_ro, xw_ro_b]).transpose(0, 2, 1)
    dtwm = np.stack([f("dtproj_w"), f("dtproj_w_b")]).transpose(0, 2, 1)
    dtbm = np.stack([f("dtproj_b"), f("dtproj_b_b")])[..., None]
    shared = {
        "w_in": np.ascontiguousarray(f("in_proj_w").T),
        "convw": np.ascontiguousarray(
            cw.reshape(2, 2, 128, 4).transpose(2, 0, 1, 3)),
        "convb": np.ascontiguousarray(
            cb.reshape(2, 2, 128, 1).transpose(2, 0, 1, 3)),
        "xw": np.ascontiguousarray(
            xwm.reshape(2, 2, 128, 40).transpose(2, 0, 1, 3)),
        "dtw": np.ascontiguousarray(dtwm.transpose(1, 0, 2)),       # [8,2,256]
        "dtb": np.ascontiguousarray(
            dtbm.reshape(2, 2, 128, 1).transpose(2, 0, 1, 3)),
        "dpc": np.ascontiguousarray(
            np.stack([f("Dp"), f("Dp_b")])[..., None]
            .reshape(2, 2, 128, 1).transpose(2, 0, 1, 3)),
        "wout": np.ascontiguousarray(
            f("out_proj_w").T.reshape(2, 128, 128).transpose(1, 0, 2)),
        "ln1g": f("ln1_g").reshape(C, 1),
        "ln1b": f("ln1_b").reshape(C, 1),
        "ln2g": f("ln2_g").reshape(C, 1),
        "ln2b": f("ln2_b").reshape(C, 1),
    }
    return xin, shared, a_pow


def _unshard(core_outs):
    y = np.stack(core_outs)                       # [8, C, BC, T]
    y = y.transpose(0, 2, 3, 1).reshape(BSEQ, T, C)[:B * N]
    return np.ascontiguousarray(
        y.reshape(B, N, T, C).transpose(0, 2, 1, 3))


_CACHE = {}


def kernel(_trace=False, **inputs):
    xin, shared, a_pow = _prep(inputs)
    if "prog" not in _CACHE:
        _CACHE["prog"] = build_program(a_pow)
    nc = _CACHE["prog"]
    in_maps = [dict(shared, xin=xin[i]) for i in range(NCORES)]
    res = run_bass_kernel_spmd(nc, in_maps, core_ids=list(range(NCORES)),
                               trace=_trace)
    out = _unshard([r["out"] for r in res.results])
    if _trace:
        kernel.last_results = res
    return out



# revision 3
# speedup vs baseline: 1.1670x; 1.1670x over previous
"""BiMamba (bimamba_type='v2') Trainium2 Bass kernel.

Data-parallel over the fused B*N=828 (padded to 832) sequence axis across 8
NeuronCores (104 sequences/core). Per-core device program:
  - channels d (d_inner=256) -> 2 partition tiles of 128
  - scan-phase tensors laid out [p=d-tile, (branch, n_state, batch, time)],
    time innermost & contiguous; the selective scan runs as one DVE
    tensor_tensor_scan per (d-tile, chunk); dA is zeroed at t=0 so the
    recurrence resets at every (branch, n, sequence) segment boundary.
  - activation tables restricted to {natural_log_exp_and_others,
    silu_and_others} so exp/ln share one table (softplus = ln(1+exp),
    rstd = exp(-0.5*ln(var+eps))) -> 2 table loads per chunk instead of 14.
  - matmuls run on bf16 operands (4x PE throughput vs fp32 mode); PSUM
    accumulation stays fp32.
  - PSUM->SBUF copies, casts and activations run on the Act engine; the DVE
    only does tensor_tensor/STT/scan work, mostly in bf16 (2x mode).
"""

import numpy as np
import ml_dtypes

import concourse.bass as bass
import concourse.tile as tile
from concourse import bacc, mybir
from concourse.bass_utils import run_bass_kernel_spmd

F32 = mybir.dt.float32
BF16 = mybir.dt.bfloat16
AF = mybir.ActivationFunctionType
ALU = mybir.AluOpType

B, T, N, C = 4, 24, 207, 128
DI = 256
DS = 16
RK = 8
EPS = 1e-5
NCORES = 8
BSEQ = 832                   # padded B*N
BC = BSEQ // NCORES          # 104 sequences per core
NCHUNK = 8
CB = BC // NCHUNK            # 13 seqs per chunk
CBT = CB * T                 # 312 tokens per chunk

BF = ml_dtypes.bfloat16


def _pbcast(ap, parts=128):
    """DRAM-source AP for a DMA that replicates data across `parts`
    partitions (prepends a 0-stride partition dim)."""
    a = [[0, parts]] + [list(x) for x in ap.ap]
    return bass.AP(tensor=ap.tensor, offset=ap.offset, ap=a)


def _rev_t(ap):
    """Reverse the last free dim of an AP."""
    a = [list(x) for x in ap.ap]
    st, ct = a[-1]
    off = ap.offset + st * (ct - 1)
    a[-1] = [-st, ct]
    return bass.AP(tensor=ap.tensor, offset=off, ap=a)


def _zstride(ap, dim, count):
    """Insert a 0-stride free dim at position `dim` (0 = just after the
    partition dim)."""
    a = [list(x) for x in ap.ap]
    a.insert(1 + dim, [0, count])
    return bass.AP(tensor=ap.tensor, offset=ap.offset, ap=a)


def build_program(a_pow):
    """a_pow: 16 floats = A[0, :] (decay coeffs, d-independent, both
    branches identical; asserted on host)."""
    # Restrict the activation-table sets so ln/exp live in ONE table
    # (softplus + LN rstd + dA all use it) and silu in the other: the
    # auto-placed ACT_TABLE_LOADs drop from ~14 to 2 per chunk.
    _gat = bacc.get_activation_tables

    def _patched(arch):
        # Keep every entry (act_func_set_id is positional — it must match
        # act_info.json's ordering) but empty the sets we don't want so the
        # placement pass can only pick the two we do.
        t = _gat(arch)
        keep = ("natural_log_exp_and_others", "silu_and_others")
        return {k: (v if k in keep else set()) for k, v in t.items()}

    bacc.get_activation_tables = _patched
    try:
        return _build_program(a_pow)
    finally:
        bacc.get_activation_tables = _gat


def _build_program(a_pow):
    nc = bacc.Bacc("TRN2", target_bir_lowering=False, debug=False,
                   enable_asserts=False, num_devices=NCORES)

    def din(name, shape, dt=F32):
        return nc.dram_tensor(name, shape, dt, kind="ExternalInput").ap()

    xin = din("xin", [C, BC, T])
    w_in = din("w_in", [C, 4 * C], BF16)
    convw = din("convw", [128, 2, 2, 4])
    convb = din("convb", [128, 2, 2, 1])
    xw = din("xw", [128, 2, 2, 40], BF16)
    dtw = din("dtw", [RK, 2, DI], BF16)
    dtb = din("dtb", [128, 2, 2, 1])
    dpc = din("dpc", [128, 2, 2, 1])
    wout = din("wout", [128, 2, C], BF16)
    ln1g = din("ln1g", [C, 1])
    ln1b = din("ln1b", [C, 1])
    ln2g = din("ln2g", [C, 1])
    ln2b = din("ln2b", [C, 1])
    out = nc.dram_tensor("out", [C, BC, T], F32, kind="ExternalOutput").ap()

    with tile.TileContext(nc) as tc, \
         tc.tile_pool(name="weights", bufs=1) as wp, \
         tc.tile_pool(name="small", bufs=2) as sp, \
         tc.tile_pool(name="stats", bufs=2) as stp, \
         tc.tile_pool(name="big", bufs=2) as bp, \
         tc.tile_pool(name="bcrep", bufs=1) as bcp, \
         tc.tile_pool(name="dram", bufs=2, space="DRAM") as drp, \
         tc.tile_pool(name="psA", bufs=2, space="PSUM") as psA, \
         tc.tile_pool(name="psB", bufs=1, space="PSUM") as psB, \
         tc.tile_pool(name="psC", bufs=2, space="PSUM") as psC:

        def load_w(name, ap_src, shape, dt=F32):
            t = wp.tile(shape, dt, tag=name, name=name)
            nc.sync.dma_start(t[:], ap_src)
            return t

        w_in_sb = load_w("w_in", w_in, [C, 4 * C], BF16)
        convw_sb = load_w("convw", convw, [128, 2, 2, 4])
        convb_sb = load_w("convb", convb, [128, 2, 2, 1])
        xw_sb = load_w("xw", xw, [128, 2, 2, 40], BF16)
        dtw_sb = load_w("dtw", dtw, [RK, 2, DI], BF16)
        dtb_sb = load_w("dtb", dtb, [128, 2, 2, 1])
        dpc_sb = load_w("dpc", dpc, [128, 2, 2, 1])
        wout_sb = load_w("wout", wout, [128, 2, C], BF16)
        ln1g_sb = load_w("ln1g", ln1g, [C, 1])
        ln1b_sb = load_w("ln1b", ln1b, [C, 1])
        ln2g_sb = load_w("ln2g", ln2g, [C, 1])
        ln2b_sb = load_w("ln2b", ln2b, [C, 1])
        ones_bf = wp.tile([C, 1], BF16, tag="ones_bf")
        nc.vector.memset(ones_bf[:], 1.0)
        eps_sb = wp.tile([C, 1], F32, tag="eps")
        nc.vector.memset(eps_sb[:], EPS)
        ones_row_bf = wp.tile([1, C], BF16, tag="ones_row_bf")
        nc.vector.memset(ones_row_bf[:], 1.0)

        def layernorm(src_f32, src_bf, sq_bf, g_sb, b_sb, dst):
            """LN over the partition (channel) dim -> dst.
            src_f32: f32 [C, CBT] for the apply path; src_bf/sq_bf: bf16
            [C, CBT] views of src and src^2 for the PE stat sums."""
            ps_s = psC.tile([1, CBT], F32, tag="ps_stat", name="ps_s")
            ps_q = psC.tile([1, CBT], F32, tag="ps_stat", name="ps_q")
            nc.tensor.matmul(ps_s[:], ones_bf[:], src_bf, start=True, stop=True)
            nc.tensor.matmul(ps_q[:], ones_bf[:], sq_bf, start=True, stop=True)
            mean = stp.tile([1, CBT], F32, tag="mean")
            nc.vector.tensor_scalar(mean[:], ps_s[:], 1.0 / C, None, ALU.mult)
            var = stp.tile([1, CBT], F32, tag="var")
            nc.vector.tensor_scalar(var[:], ps_q[:], 1.0 / C, None, ALU.mult)
            m2 = stp.tile([1, CBT], F32, tag="m2")
            nc.vector.tensor_mul(m2[:], mean[:], mean[:])
            nc.vector.tensor_sub(var[:], var[:], m2[:])
            # rstd = (var+eps)^-0.5 = exp(-0.5*ln(var+eps)); no sqrt table
            nc.scalar.activation(var[:], var[:], AF.Ln, bias=eps_sb[0:1, 0:1])
            nc.scalar.activation(var[:], var[:], AF.Exp, scale=-0.5)
            mean_bf = stp.tile([1, CBT], BF16, tag="mean_bf")
            nc.scalar.copy(mean_bf[:], mean[:])
            rstd_bf = stp.tile([1, CBT], BF16, tag="rstd_bf")
            nc.scalar.copy(rstd_bf[:], var[:])
            mean_r = psC.tile([C, CBT], F32, tag="ps_stat", name="mean_r")
            nc.tensor.matmul(mean_r[:], ones_row_bf[:], mean_bf[:],
                             start=True, stop=True)
            rstd_r = psC.tile([C, CBT], F32, tag="ps_stat", name="rstd_r")
            nc.tensor.matmul(rstd_r[:], ones_row_bf[:], rstd_bf[:],
                             start=True, stop=True)
            tln = sp.tile([C, CBT], F32, tag="ln_tmp")
            nc.vector.tensor_sub(tln[:], src_f32, mean_r[:])
            nc.vector.tensor_mul(tln[:], tln[:], rstd_r[:])
            nc.vector.tensor_scalar(dst, tln[:], g_sb[:, 0:1], b_sb[:, 0:1],
                                    ALU.mult, ALU.add)

        for ch in range(NCHUNK):
            b0 = ch * CB
            u = sp.tile([C, CB, T], F32, tag="u")
            nc.sync.dma_start(u[:], xin[:, b0:b0 + CB, :])
            uf = u[:].rearrange("p b t -> p (b t)")

            # LN1 (stat sums on PE from bf16 casts; apply on DVE in f32)
            u_bf = sp.tile([C, CBT], BF16, tag="u_bf")
            nc.scalar.copy(u_bf[:], uf)
            sq_bf = sp.tile([C, CBT], BF16, tag="sq_bf")
            nc.scalar.square(sq_bf[:], uf)
            hln = sp.tile([C, CBT], BF16, tag="hln")
            layernorm(uf, u_bf[:], sq_bf[:], ln1g_sb, ln1b_sb, hln[:])

            # in_proj (m-tiles: xx0 xx1 z0 z1); copies/silu on Act engine
            xx = [sp.tile([128, CB, T], BF16, tag=f"xx{ti}", name=f"xx{ti}")
                  for ti in range(2)]
            sz = [sp.tile([128, CB, T], BF16, tag=f"sz{ti}", name=f"sz{ti}")
                  for ti in range(2)]
            for mt in range(4):
                ps_xz = psA.tile([128, CBT], F32, tag="ps_xz")
                nc.tensor.matmul(ps_xz[:], w_in_sb[:, mt * 128:(mt + 1) * 128],
                                 hln[:], start=True, stop=True)
                dst = xx[mt][:] if mt < 2 else sz[mt - 2][:]
                pv = ps_xz[:].rearrange("p (b t) -> p b t", t=T)
                if mt < 2:
                    nc.scalar.copy(dst, pv)
                else:
                    nc.scalar.activation(dst, pv, AF.Silu)

            # causal depthwise conv (DVE, bf16) + silu (Act)
            xc2 = [sp.tile([128, 2, CB, T], BF16, tag=f"xc{ti}",
                           name=f"xc{ti}", bufs=1)
                   for ti in range(2)]
            for ti in range(2):
                xxv = xx[ti][:]
                for br in range(2):
                    acc = xc2[ti][:, br, :, :]
                    w3 = convw_sb[:, br, ti, 3:4]
                    src3 = xxv if br == 0 else _rev_t(xxv)
                    nc.vector.tensor_scalar(acc, src3, w3, None, ALU.mult)
                    for k in range(3):
                        src = xxv[:, :, :T - (3 - k)] if br == 0 \
                            else _rev_t(xxv[:, :, 3 - k:])
                        nc.vector.scalar_tensor_tensor(
                            acc[:, :, 3 - k:], src,
                            convw_sb[:, br, ti, k:k + 1],
                            acc[:, :, 3 - k:], ALU.mult, ALU.add)
                    nc.scalar.activation(acc, acc, AF.Silu,
                                         bias=convb_sb[:, br, ti, 0:1])

            # xproj -> x_dbl [40, CBT] per branch; stage dtraw + B/C bf16
            dtraw, bc_bf = [None, None], [None, None]
            for br in range(2):
                ps_xd = psB.tile([40, CBT], F32, tag="ps_xd")
                for ti in range(2):
                    nc.tensor.matmul(ps_xd[:], xw_sb[:, br, ti, :],
                                     xc2[ti][:, br, :, :].rearrange(
                                         "p b t -> p (b t)"),
                                     start=(ti == 0), stop=(ti == 1))
                bc_bf[br] = stp.tile([32, CBT], BF16, tag=f"bcbf{br}",
                                     name=f"bcbf{br}")
                nc.scalar.copy(bc_bf[br][:], ps_xd[0:32, :])
                dtraw[br] = stp.tile([RK, CBT], BF16, tag=f"dtraw{br}",
                                     name=f"dtraw{br}")
                nc.scalar.copy(dtraw[br][:], ps_xd[32:40, :])

            # B_rep / C_rep via DRAM round-trip broadcast
            b1d = drp.tile([2, DS, CB, T], BF16, tag="b1d")
            c1d = drp.tile([2, DS, CB, T], BF16, tag="c1d")
            for br in range(2):
                nc.sync.dma_start(b1d[br, :, :, :],
                                  bc_bf[br][0:DS, :].rearrange(
                                      "p (b t) -> p b t", t=T))
                nc.sync.dma_start(c1d[br, :, :, :],
                                  bc_bf[br][DS:32, :].rearrange(
                                      "p (b t) -> p b t", t=T))
                # (bc_bf rows: 0:16 = B, 16:32 = C; dtraw from rows 32:40)
            brep = bcp.tile([128, 2 * DS * CBT], BF16, tag="brep")
            nc.sync.dma_start(
                brep[:], _pbcast(b1d[:].rearrange("a n b t -> (a n b t)")))
            crep = bcp.tile([128, 2 * DS * CBT], BF16, tag="crep")
            nc.sync.dma_start(
                crep[:], _pbcast(c1d[:].rearrange("a n b t -> (a n b t)")))
            brep5 = brep[:].rearrange("p (a n b t) -> p a n b t",
                                      a=2, n=DS, b=CB)
            crep5 = crep[:].rearrange("p (a n b t) -> p a n b t",
                                      a=2, n=DS, b=CB)

            # dtproj; dt = ln(1 + exp(x + bias)); bf16 out, both acts on
            # the shared ln/exp table
            dt2 = [sp.tile([128, 2, CB, T], BF16, tag=f"dt{ti}",
                           name=f"dt{ti}", bufs=1)
                   for ti in range(2)]
            for br in range(2):
                for ti in range(2):
                    ps_dt = psB.tile([128, CBT], F32, tag="ps_dt")
                    nc.tensor.matmul(ps_dt[:],
                                     dtw_sb[:, br, ti * 128:(ti + 1) * 128],
                                     dtraw[br][:], start=True, stop=True)
                    slab = dt2[ti][:, br, :, :]
                    nc.scalar.activation(
                        slab, ps_dt[:].rearrange("p (b t) -> p b t", t=T),
                        AF.Exp, bias=dtb_sb[:, br, ti, 0:1])
                    nc.scalar.activation(slab, slab, AF.Ln, bias=1.0)

            ps_o = psB.tile([128, CBT], F32, tag="ps_o")
            for ti in range(2):
                # du = dt * xc  (bf16 2x)
                du2 = bp.tile([128, 2, CB, T], BF16, tag="du2")
                nc.vector.tensor_mul(du2[:], dt2[ti][:], xc2[ti][:])

                # dA[n] = exp(a_n * dt); zero at t=0 (scan segment reset)
                dA = bp.tile([128, 2, DS, CB, T], BF16, tag="dA")
                for n in range(DS):
                    nc.scalar.activation(dA[:, :, n, :, :], dt2[ti][:],
                                         AF.Exp, scale=float(a_pow[n]))
                nc.gpsimd.memset(dA[:, :, :, :, 0:1], 0.0)

                # dBu = du (bcast over n) * B_rep
                dBu = bp.tile([128, 2, DS, CB, T], BF16, tag="dBu")
                nc.vector.tensor_mul(dBu[:], _zstride(du2[:], 1, DS), brep5)

                # selective scan along (br, n, b, t) flattened free axis
                h = bp.tile([128, 2, DS, CB, T], BF16, tag="h", bufs=1)
                nc.vector.tensor_tensor_scan(
                    h[:].rearrange("p a n b t -> p (a n b t)"),
                    dA[:].rearrange("p a n b t -> p (a n b t)"),
                    dBu[:].rearrange("p a n b t -> p (a n b t)"),
                    0.0, ALU.mult, ALU.add)

                # hc = h * C_rep; tree-reduce over n into the n=0 slab
                nc.vector.tensor_mul(h[:], h[:], crep5)
                for w in (8, 4, 2, 1):
                    nc.vector.tensor_add(h[:, :, 0:w, :, :],
                                         h[:, :, 0:w, :, :],
                                         h[:, :, w:2 * w, :, :])

                # per-branch y_br = scan_out + Dp_br*xc_br (branch coords),
                # then y = y_f + rev(y_b), gated by silu(z)
                yf = sp.tile([128, CB, T], BF16, tag=f"yf{ti}", name=f"yf{ti}")
                nc.vector.scalar_tensor_tensor(
                    yf[:], xc2[ti][:, 0, :, :], dpc_sb[:, 0, ti, 0:1],
                    h[:, 0, 0, :, :], ALU.mult, ALU.add)
                yb = sp.tile([128, CB, T], BF16, tag=f"yb{ti}", name=f"yb{ti}")
                nc.vector.scalar_tensor_tensor(
                    yb[:], xc2[ti][:, 1, :, :], dpc_sb[:, 1, ti, 0:1],
                    h[:, 1, 0, :, :], ALU.mult, ALU.add)
                ypre = sp.tile([128, CB, T], BF16, tag=f"ypre{ti}",
                               name=f"ypre{ti}")
                nc.vector.tensor_add(ypre[:], yf[:], _rev_t(yb[:]))
                nc.vector.tensor_mul(ypre[:], ypre[:], sz[ti][:])

                nc.tensor.matmul(ps_o[:], wout_sb[:, ti, :],
                                 ypre[:].rearrange("p b t -> p (b t)"),
                                 start=(ti == 0), stop=(ti == 1))

            # LN2 + residual
            o_sb = sp.tile([C, CBT], F32, tag="o_sb")
            nc.scalar.copy(o_sb[:], ps_o[:])
            o_bf = sp.tile([C, CBT], BF16, tag="o_bf")
            nc.scalar.copy(o_bf[:], ps_o[:])
            sq2_bf = sp.tile([C, CBT], BF16, tag="sq2_bf")
            nc.scalar.square(sq2_bf[:], ps_o[:])
            o_ln = sp.tile([C, CBT], F32, tag="o_ln")
            layernorm(o_sb[:], o_bf[:], sq2_bf[:], ln2g_sb, ln2b_sb, o_ln[:])
            nc.vector.tensor_add(o_ln[:], o_ln[:], uf)
            nc.sync.dma_start(out[:, b0:b0 + CB, :],
                              o_ln[:].rearrange("p (b t) -> p b t", t=T))

    nc.finalize()
    return nc


def _prep(inputs):
    f = lambda k: np.ascontiguousarray(np.asarray(inputs[k], np.float32))
    bf = lambda a: np.ascontiguousarray(np.asarray(a, BF))
    x = f("x")
    u_all = x.transpose(0, 2, 1, 3).reshape(B * N, T, C)
    u_pad = np.zeros((BSEQ, T, C), np.float32)
    u_pad[:B * N] = u_all
    xin = [np.ascontiguousarray(u_pad[i * BC:(i + 1) * BC].transpose(2, 0, 1))
           for i in range(NCORES)]

    A = -np.exp(f("A_log"))
    Ab = -np.exp(f("A_b_log"))
    assert np.allclose(A, A[0:1], rtol=1e-5), "A must be d-independent"
    assert np.allclose(Ab, A, rtol=1e-5), "A_b must equal A"
    a_pow = [float(v) for v in A[0]]

    cw = np.stack([f("conv_w")[:, 0, :], f("conv_w_b")[:, 0, :]])   # [2,256,4]
    cb = np.stack([f("conv_b"), f("conv_b_b")])[..., None]          # [2,256,1]
    xw_ro = np.concatenate([f("xproj_w")[RK:], f("xproj_w")[:RK]])
    xw_ro_b = np.concatenate([f("xproj_w_b")[RK:], f("xproj_w_b")[:RK]])
    xwm = np.stack([xw_ro, xw_ro_b]).transpose(0, 2, 1)
    dtwm = np.stack([f("dtproj_w"), f("dtproj_w_b")]).transpose(0, 2, 1)
    dtbm = np.stack([f("dtproj_b"), f("dtproj_b_b")])[..., None]
    shared = {
        "w_in": bf(f("in_proj_w").T),
        "convw": np.ascontiguousarray(
            cw.reshape(2, 2, 128, 4).transpose(2, 0, 1, 3)),
        "convb": np.ascontiguousarray(
            cb.reshape(2, 2, 128, 1).transpose(2, 0, 1, 3)),
        "xw": bf(xwm.reshape(2, 2, 128, 40).transpose(2, 0, 1, 3)),
        "dtw": bf(dtwm.transpose(1, 0, 2)),                         # [8,2,256]
        "dtb": np.ascontiguousarray(
            dtbm.reshape(2, 2, 128, 1).transpose(2, 0, 1, 3)),
        "dpc": np.ascontiguousarray(
            np.stack([f("Dp"), f("Dp_b")])[..., None]
            .reshape(2, 2, 128, 1).transpose(2, 0, 1, 3)),
        "wout": bf(f("out_proj_w").T.reshape(2, 128, 128).transpose(1, 0, 2)),
        "ln1g": f("ln1_g").reshape(C, 1),
        "ln1b": f("ln1_b").reshape(C, 1),
        "ln2g": f("ln2_g").reshape(C, 1),
        "ln2b": f("ln2_b").reshape(C, 1),
    }
    return xin, shared, a_pow


def _unshard(core_outs):
    y = np.stack(core_outs)                       # [8, C, BC, T]
    y = y.transpose(0, 2, 3, 1).reshape(BSEQ, T, C)[:B * N]
    return np.ascontiguousarray(
        y.reshape(B, N, T, C).transpose(0, 2, 1, 3))


_CACHE = {}


def kernel(_trace=False, **inputs):
    xin, shared, a_pow = _prep(inputs)
    if "prog" not in _CACHE:
        _CACHE["prog"] = build_program(a_pow)
    nc = _CACHE["prog"]
    in_maps = [dict(shared, xin=xin[i]) for i in range(NCORES)]
    res = run_bass_kernel_spmd(nc, in_maps, core_ids=list(range(NCORES)),
                               trace=_trace)
    out = _unshard([r["out"] for r in res.results])
    if _trace:
        kernel.last_results = res
    return out


# revision 6
# speedup vs baseline: 1.3981x; 1.1981x over previous
"""BiMamba (bimamba_type='v2') Trainium2 Bass kernel.

Data-parallel over the fused B*N=828 (padded to 832) sequence axis across 8
NeuronCores (104 sequences/core). Per-core device program:
  - channels d (d_inner=256) -> 2 partition tiles of 128
  - scan-phase tensors laid out [p=d-tile, (n_state, branch, batch, time)]
    (n-major so each dA slab and the n-tree-reduce slices are contiguous);
    the selective scan runs as two DVE tensor_tensor_scan per (d-tile,
    chunk): branch 0 forward, branch 1 via time-reversed APs (all branch-1
    tensors are kept forward-oriented; only the scan walks t backwards).
    dA is zeroed at the per-(n,seq) segment start (t=0 fwd / t=T-1 bwd).
  - depthwise causal conv runs on the PE as 4 shifted diagonal matmuls
    accumulating in PSUM (branch 1 uses the anti-causal shifts).
  - activation tables restricted to {natural_log_exp_and_others,
    silu_and_others}: softplus = ln(1+exp), rstd = exp(-0.5*ln(var+eps)),
    so exp/ln share one table -> 2 table loads per chunk.
  - all matmuls on bf16 operands (fp32 PSUM accumulate); PSUM->SBUF
    copies/casts/activations on the Act engine; DVE does only TT/STT/scan,
    bf16 (2x) where possible.
  - chunk loop is software-pipelined: front(c+1) (LN1/in_proj/conv/xproj/
    dtproj issue) is emitted before back(c) (scan section) so the Act/PE
    latency chain of the next chunk hides under the current chunk's DVE
    scan work.
"""

import numpy as np
import ml_dtypes

import concourse.bass as bass
import concourse.tile as tile
from concourse import bacc, mybir
from concourse.bass_utils import run_bass_kernel_spmd

F32 = mybir.dt.float32
BF16 = mybir.dt.bfloat16
AF = mybir.ActivationFunctionType
ALU = mybir.AluOpType

B, T, N, C = 4, 24, 207, 128
DI = 256
DS = 16
RK = 8
EPS = 1e-5
NCORES = 8
BSEQ = 832                   # padded B*N
BC = BSEQ // NCORES          # 104 sequences per core
NCHUNK = 8
CB = BC // NCHUNK            # 13 seqs per chunk
CBT = CB * T                 # 312 tokens per chunk

BF = ml_dtypes.bfloat16


def _pbcast(ap, parts=128):
    """DRAM-source AP for a DMA that replicates data across `parts`
    partitions (prepends a 0-stride partition dim)."""
    a = [[0, parts]] + [list(x) for x in ap.ap]
    return bass.AP(tensor=ap.tensor, offset=ap.offset, ap=a)


def _rev_t(ap):
    """Reverse the last free dim of an AP."""
    a = [list(x) for x in ap.ap]
    st, ct = a[-1]
    off = ap.offset + st * (ct - 1)
    a[-1] = [-st, ct]
    return bass.AP(tensor=ap.tensor, offset=off, ap=a)


def _zstride(ap, dim, count):
    """Insert a 0-stride free dim at position `dim` (0 = just after the
    partition dim)."""
    a = [list(x) for x in ap.ap]
    a.insert(1 + dim, [0, count])
    return bass.AP(tensor=ap.tensor, offset=ap.offset, ap=a)


def build_program(a_pow):
    """a_pow: 16 floats = A[0, :] (decay coeffs, d-independent, both
    branches identical; asserted on host)."""
    # Restrict the activation-table sets so ln/exp live in ONE table and
    # silu in the other (act_func_set_id is positional, so keep every
    # entry but empty the unwanted sets).
    _gat = bacc.get_activation_tables

    def _patched(arch):
        t = _gat(arch)
        keep = ("natural_log_exp_and_others", "silu_and_others")
        return {k: (v if k in keep else set()) for k, v in t.items()}

    bacc.get_activation_tables = _patched
    try:
        return _build_program(a_pow)
    finally:
        bacc.get_activation_tables = _gat


def _build_program(a_pow):
    nc = bacc.Bacc("TRN2", target_bir_lowering=False, debug=False,
                   enable_asserts=False, num_devices=NCORES)

    def din(name, shape, dt=F32):
        return nc.dram_tensor(name, shape, dt, kind="ExternalInput").ap()

    xin = din("xin", [C, BC, T])
    w_in = din("w_in", [C, 4 * C], BF16)
    convd = din("convd", [128, 2, 2, 4, 128], BF16)   # diag conv weights
    convb = din("convb", [128, 2, 2, 1])
    xw = din("xw", [128, 2, 2, 40], BF16)
    dtw = din("dtw", [RK, 2, DI], BF16)
    dtb = din("dtb", [128, 2, 2, 1])
    dpc = din("dpc", [128, 2, 2, 1])
    wout = din("wout", [128, 2, C], BF16)
    ln1g = din("ln1g", [C, 1])
    ln1b = din("ln1b", [C, 1])
    ln2g = din("ln2g", [C, 1])
    ln2b = din("ln2b", [C, 1])
    out = nc.dram_tensor("out", [C, BC, T], F32, kind="ExternalOutput").ap()

    with tile.TileContext(nc) as tc, \
         tc.tile_pool(name="weights", bufs=1) as wp, \
         tc.tile_pool(name="small", bufs=2) as sp, \
         tc.tile_pool(name="stats", bufs=2) as stp, \
         tc.tile_pool(name="big", bufs=2) as bp, \
         tc.tile_pool(name="bcrep", bufs=1) as bcp, \
         tc.tile_pool(name="dram", bufs=2, space="DRAM") as drp, \
         tc.tile_pool(name="psA", bufs=2, space="PSUM") as psA, \
         tc.tile_pool(name="psB", bufs=1, space="PSUM") as psB, \
         tc.tile_pool(name="psC", bufs=2, space="PSUM") as psC:

        def load_w(name, ap_src, shape, dt=F32):
            t = wp.tile(shape, dt, tag=name, name=name)
            nc.sync.dma_start(t[:], ap_src)
            return t

        w_in_sb = load_w("w_in", w_in, [C, 4 * C], BF16)
        convd_sb = load_w("convd", convd, [128, 2, 2, 4, 128], BF16)
        convb_sb = load_w("convb", convb, [128, 2, 2, 1])
        xw_sb = load_w("xw", xw, [128, 2, 2, 40], BF16)
        dtw_sb = load_w("dtw", dtw, [RK, 2, DI], BF16)
        dtb_sb = load_w("dtb", dtb, [128, 2, 2, 1])
        dpc_sb = load_w("dpc", dpc, [128, 2, 2, 1])
        wout_sb = load_w("wout", wout, [128, 2, C], BF16)
        ln1g_sb = load_w("ln1g", ln1g, [C, 1])
        ln1b_sb = load_w("ln1b", ln1b, [C, 1])
        ln2g_sb = load_w("ln2g", ln2g, [C, 1])
        ln2b_sb = load_w("ln2b", ln2b, [C, 1])
        ones_bf = wp.tile([C, 1], BF16, tag="ones_bf")
        nc.vector.memset(ones_bf[:], 1.0)
        eps_sb = wp.tile([C, 1], F32, tag="eps")
        nc.vector.memset(eps_sb[:], EPS)
        ones_row_bf = wp.tile([1, C], BF16, tag="ones_row_bf")
        nc.vector.memset(ones_row_bf[:], 1.0)

        def layernorm(src_f32, src_bf, sq_bf, g_sb, b_sb, dst):
            """LN over the partition (channel) dim -> dst.
            src_f32 for the apply path; src_bf/sq_bf: bf16 casts of src and
            src^2 for the PE stat sums."""
            ps_s = psC.tile([1, CBT], F32, tag="ps_stat", name="ps_s")
            ps_q = psC.tile([1, CBT], F32, tag="ps_stat", name="ps_q")
            nc.tensor.matmul(ps_s[:], ones_bf[:], src_bf, start=True, stop=True)
            nc.tensor.matmul(ps_q[:], ones_bf[:], sq_bf, start=True, stop=True)
            mean = stp.tile([1, CBT], F32, tag="mean")
            nc.vector.tensor_scalar(mean[:], ps_s[:], 1.0 / C, None, ALU.mult)
            var = stp.tile([1, CBT], F32, tag="var")
            nc.vector.tensor_scalar(var[:], ps_q[:], 1.0 / C, None, ALU.mult)
            m2 = stp.tile([1, CBT], F32, tag="m2")
            nc.vector.tensor_mul(m2[:], mean[:], mean[:])
            nc.vector.tensor_sub(var[:], var[:], m2[:])
            # rstd = (var+eps)^-0.5 = exp(-0.5*ln(var+eps)); no sqrt table
            nc.scalar.activation(var[:], var[:], AF.Ln, bias=eps_sb[0:1, 0:1])
            nc.scalar.activation(var[:], var[:], AF.Exp, scale=-0.5)
            mean_bf = stp.tile([1, CBT], BF16, tag="mean_bf")
            nc.scalar.copy(mean_bf[:], mean[:])
            rstd_bf = stp.tile([1, CBT], BF16, tag="rstd_bf")
            nc.scalar.copy(rstd_bf[:], var[:])
            mean_r = psC.tile([C, CBT], F32, tag="ps_stat", name="mean_r")
            nc.tensor.matmul(mean_r[:], ones_row_bf[:], mean_bf[:],
                             start=True, stop=True)
            rstd_r = psC.tile([C, CBT], F32, tag="ps_stat", name="rstd_r")
            nc.tensor.matmul(rstd_r[:], ones_row_bf[:], rstd_bf[:],
                             start=True, stop=True)
            tln = sp.tile([C, CBT], F32, tag="ln_tmp")
            nc.vector.tensor_sub(tln[:], src_f32, mean_r[:])
            nc.vector.tensor_mul(tln[:], tln[:], rstd_r[:])
            nc.vector.tensor_scalar(dst, tln[:], g_sb[:, 0:1], b_sb[:, 0:1],
                                    ALU.mult, ALU.add)

        def front(ch):
            """LN1 + in_proj + conv(PE) + xproj + B/C broadcast + dtproj.
            Returns the tiles back(ch) needs."""
            b0 = ch * CB
            u = sp.tile([C, CB, T], F32, tag="u")
            nc.sync.dma_start(u[:], xin[:, b0:b0 + CB, :])
            uf = u[:].rearrange("p b t -> p (b t)")

            u_bf = sp.tile([C, CBT], BF16, tag="u_bf")
            nc.scalar.copy(u_bf[:], uf)
            sq_bf = sp.tile([C, CBT], BF16, tag="sq_bf")
            nc.scalar.square(sq_bf[:], uf)
            hln = sp.tile([C, CBT], BF16, tag="hln")
            layernorm(uf, u_bf[:], sq_bf[:], ln1g_sb, ln1b_sb, hln[:])

            # in_proj (m-tiles: xx0 xx1 z0 z1); copies/silu on Act engine
            xx = [sp.tile([128, CB, T], BF16, tag=f"xx{ti}", name=f"xx{ti}")
                  for ti in range(2)]
            sz = [sp.tile([128, CB, T], BF16, tag=f"sz{ti}", name=f"sz{ti}")
                  for ti in range(2)]
            for mt in range(4):
                ps_xz = psA.tile([128, CBT], F32, tag="ps_xz")
                nc.tensor.matmul(ps_xz[:], w_in_sb[:, mt * 128:(mt + 1) * 128],
                                 hln[:], start=True, stop=True)
                dst = xx[mt][:] if mt < 2 else sz[mt - 2][:]
                pv = ps_xz[:].rearrange("p (b t) -> p b t", t=T)
                if mt < 2:
                    nc.scalar.copy(dst, pv)
                else:
                    nc.scalar.activation(dst, pv, AF.Silu)

            # depthwise conv on PE: 4 shifted diagonal matmuls per (ti, br).
            # br=0 causal: ps[:, :, 3-k:] += Wk . xx[:, :, :T-(3-k)]
            # br=1 anti-causal (forward-oriented backward branch):
            #   ps[:, :, :T-(3-k)] += Wk . xx[:, :, 3-k:]
            # then silu(+bias) Act reads PSUM -> xc2 bf16.
            xc2 = [sp.tile([128, 2, CB, T], BF16, tag=f"xc{ti}",
                           name=f"xc{ti}")
                   for ti in range(2)]
            for ti in range(2):
                xxv = xx[ti][:]
                for br in range(2):
                    ps_cv = psA.tile([128, CB, T], F32, tag="ps_xz",
                                     name=f"ps_cv{ti}{br}")
                    nc.tensor.matmul(ps_cv[:], convd_sb[:, br, ti, 3, :],
                                     xxv, start=True, stop=False)
                    for k in range(3):
                        s = 3 - k
                        if br == 0:
                            dst, src = ps_cv[:, :, s:], xxv[:, :, :T - s]
                        else:
                            dst, src = ps_cv[:, :, :T - s], xxv[:, :, s:]
                        nc.tensor.matmul(dst, convd_sb[:, br, ti, k, :], src,
                                         start=False, stop=(k == 2))
                    nc.scalar.activation(xc2[ti][:, br, :, :], ps_cv[:],
                                         AF.Silu, bias=convb_sb[:, br, ti, 0:1])

            # xproj -> x_dbl [40, CBT] per branch; stage dtraw + B/C bf16
            dtraw, bc_bf = [None, None], [None, None]
            for br in range(2):
                ps_xd = psA.tile([40, CBT], F32, tag="ps_m",
                                 name=f"ps_xd{br}")
                for ti in range(2):
                    nc.tensor.matmul(ps_xd[:], xw_sb[:, br, ti, :],
                                     xc2[ti][:, br, :, :].rearrange(
                                         "p b t -> p (b t)"),
                                     start=(ti == 0), stop=(ti == 1))
                bc_bf[br] = stp.tile([32, CBT], BF16, tag=f"bcbf{br}",
                                     name=f"bcbf{br}")
                nc.scalar.copy(bc_bf[br][:], ps_xd[0:32, :])
                dtraw[br] = stp.tile([RK, CBT], BF16, tag=f"dtraw{br}",
                                     name=f"dtraw{br}")
                nc.scalar.copy(dtraw[br][:], ps_xd[32:40, :])

            # B_rep / C_rep via DRAM round-trip broadcast (n-major layout)
            b1d = drp.tile([2, DS, CB, T], BF16, tag="b1d")
            c1d = drp.tile([2, DS, CB, T], BF16, tag="c1d")
            for br in range(2):
                nc.sync.dma_start(b1d[br, :, :, :],
                                  bc_bf[br][0:DS, :].rearrange(
                                      "p (b t) -> p b t", t=T))
                nc.sync.dma_start(c1d[br, :, :, :],
                                  bc_bf[br][DS:32, :].rearrange(
                                      "p (b t) -> p b t", t=T))
            brep = bcp.tile([128, 2 * DS * CBT], BF16, tag="brep")
            nc.sync.dma_start(
                brep[:], _pbcast(b1d[:].rearrange("a n b t -> (a n b t)")))
            crep = bcp.tile([128, 2 * DS * CBT], BF16, tag="crep")
            nc.sync.dma_start(
                crep[:], _pbcast(c1d[:].rearrange("a n b t -> (a n b t)")))

            # dtproj; dt = ln(1 + exp(x + bias)); both acts share one table
            dt2 = [sp.tile([128, 2, CB, T], BF16, tag=f"dt{ti}",
                           name=f"dt{ti}")
                   for ti in range(2)]
            for br in range(2):
                for ti in range(2):
                    ps_dt = psA.tile([128, CBT], F32, tag="ps_m",
                                     name=f"ps_dt{br}{ti}")
                    nc.tensor.matmul(ps_dt[:],
                                     dtw_sb[:, br, ti * 128:(ti + 1) * 128],
                                     dtraw[br][:], start=True, stop=True)
                    slab = dt2[ti][:, br, :, :]
                    nc.scalar.activation(
                        slab, ps_dt[:].rearrange("p (b t) -> p b t", t=T),
                        AF.Exp, bias=dtb_sb[:, br, ti, 0:1])
                    nc.scalar.activation(slab, slab, AF.Ln, bias=1.0)

            return dict(ch=ch, uf=uf, xc2=xc2, sz=sz, dt2=dt2,
                        brep=brep, crep=crep)

        def back(st):
            """Scan section + gating + out_proj + LN2 + residual + store."""
            ch, uf = st["ch"], st["uf"]
            xc2, sz, dt2 = st["xc2"], st["sz"], st["dt2"]
            b0 = ch * CB
            brep5 = st["brep"][:].rearrange("p (a n b t) -> p a n b t",
                                            a=2, n=DS, b=CB)
            crep5 = st["crep"][:].rearrange("p (a n b t) -> p a n b t",
                                            a=2, n=DS, b=CB)

            ps_o = psB.tile([128, CBT], F32, tag="ps_o")
            for ti in range(2):
                # du = dt * xc  (bf16 2x)
                du2 = bp.tile([128, 2, CB, T], BF16, tag="du2")
                nc.vector.tensor_mul(du2[:], dt2[ti][:], xc2[ti][:])

                # dA[n] = exp(a_n * dt)
                dA = bp.tile([128, 2, DS, CB, T], BF16, tag="dA")
                for n in range(DS):
                    nc.scalar.activation(dA[:, :, n, :, :], dt2[ti][:],
                                         AF.Exp, scale=float(a_pow[n]))
                # segment resets: br0 scans fwd (reset t=0), br1 scans bwd
                # (reset t=T-1)
                nc.gpsimd.memset(dA[:, 0, :, :, 0:1], 0.0)
                nc.gpsimd.memset(dA[:, 1, :, :, T - 1:T], 0.0)

                # dBu = du (bcast over n) * B_rep
                dBu = bp.tile([128, 2, DS, CB, T], BF16, tag="dBu")
                nc.vector.tensor_mul(dBu[:], _zstride(du2[:], 1, DS), brep5)

                # selective scan: branch 0 forward, branch 1 reversed-t APs
                h = bp.tile([128, 2, DS, CB, T], BF16, tag="h", bufs=1)
                flat = "p n b t -> p (n b t)"
                nc.vector.tensor_tensor_scan(
                    h[:, 0, :, :, :].rearrange(flat),
                    dA[:, 0, :, :, :].rearrange(flat),
                    dBu[:, 0, :, :, :].rearrange(flat),
                    0.0, ALU.mult, ALU.add)
                nc.vector.tensor_tensor_scan(
                    _rev_t(h[:, 1, :, :, :].rearrange(flat)),
                    _rev_t(dA[:, 1, :, :, :].rearrange(flat)),
                    _rev_t(dBu[:, 1, :, :, :].rearrange(flat)),
                    0.0, ALU.mult, ALU.add)

                # hc = h * C_rep; tree-reduce over n into the n=0 slab
                nc.vector.tensor_mul(h[:], h[:], crep5)
                for w in (8, 4, 2, 1):
                    nc.vector.tensor_add(h[:, :, 0:w, :, :],
                                         h[:, :, 0:w, :, :],
                                         h[:, :, w:2 * w, :, :])

                # y_br = scan_out + Dp_br*xc_br; both branches forward ->
                # y = y_f + y_b, gated by silu(z)
                yf = sp.tile([128, CB, T], BF16, tag=f"yf{ti}", name=f"yf{ti}")
                nc.vector.scalar_tensor_tensor(
                    yf[:], xc2[ti][:, 0, :, :], dpc_sb[:, 0, ti, 0:1],
                    h[:, 0, 0, :, :], ALU.mult, ALU.add)
                yb = sp.tile([128, CB, T], BF16, tag=f"yb{ti}", name=f"yb{ti}")
                nc.vector.scalar_tensor_tensor(
                    yb[:], xc2[ti][:, 1, :, :], dpc_sb[:, 1, ti, 0:1],
                    h[:, 1, 0, :, :], ALU.mult, ALU.add)
                ypre = sp.tile([128, CB, T], BF16, tag=f"ypre{ti}",
                               name=f"ypre{ti}")
                nc.vector.tensor_add(ypre[:], yf[:], yb[:])
                nc.vector.tensor_mul(ypre[:], ypre[:], sz[ti][:])

                nc.tensor.matmul(ps_o[:], wout_sb[:, ti, :],
                                 ypre[:].rearrange("p b t -> p (b t)"),
                                 start=(ti == 0), stop=(ti == 1))

            # LN2 + residual
            o_sb = sp.tile([C, CBT], F32, tag="o_sb")
            nc.scalar.copy(o_sb[:], ps_o[:])
            o_bf = sp.tile([C, CBT], BF16, tag="o_bf")
            nc.scalar.copy(o_bf[:], ps_o[:])
            sq2_bf = sp.tile([C, CBT], BF16, tag="sq2_bf")
            nc.scalar.square(sq2_bf[:], ps_o[:])
            o_ln = sp.tile([C, CBT], F32, tag="o_ln")
            layernorm(o_sb[:], o_bf[:], sq2_bf[:], ln2g_sb, ln2b_sb, o_ln[:])
            nc.vector.tensor_add(o_ln[:], o_ln[:], uf)
            nc.sync.dma_start(out[:, b0:b0 + CB, :],
                              o_ln[:].rearrange("p (b t) -> p b t", t=T))

        # software pipeline: front(c+1) issues while back(c)'s DVE work runs
        st = front(0)
        for ch in range(NCHUNK):
            nxt = front(ch + 1) if ch + 1 < NCHUNK else None
            back(st)
            st = nxt

    nc.finalize()
    return nc


def _prep(inputs):
    f = lambda k: np.ascontiguousarray(np.asarray(inputs[k], np.float32))
    bf = lambda a: np.ascontiguousarray(np.asarray(a, BF))
    x = f("x")
    u_all = x.transpose(0, 2, 1, 3).reshape(B * N, T, C)
    u_pad = np.zeros((BSEQ, T, C), np.float32)
    u_pad[:B * N] = u_all
    xin = [np.ascontiguousarray(u_pad[i * BC:(i + 1) * BC].transpose(2, 0, 1))
           for i in range(NCORES)]

    A = -np.exp(f("A_log"))
    Ab = -np.exp(f("A_b_log"))
    assert np.allclose(A, A[0:1], rtol=1e-5), "A must be d-independent"
    assert np.allclose(Ab, A, rtol=1e-5), "A_b must equal A"
    a_pow = [float(v) for v in A[0]]

    cw = np.stack([f("conv_w")[:, 0, :], f("conv_w_b")[:, 0, :]])   # [2,256,4]
    cb = np.stack([f("conv_b"), f("conv_b_b")])[..., None]          # [2,256,1]
    # diagonal conv weights [p, br, ti, k, m]: diag(w_k) per (br, ti)
    convd = np.zeros((128, 2, 2, 4, 128), np.float32)
    rng = np.arange(128)
    for br in range(2):
        for ti in range(2):
            for k in range(4):
                convd[rng, br, ti, k, rng] = cw[br, ti * 128:(ti + 1) * 128, k]
    xw_ro = np.concatenate([f("xproj_w")[RK:], f("xproj_w")[:RK]])
    xw_ro_b = np.concatenate([f("xproj_w_b")[RK:], f("xproj_w_b")[:RK]])
    xwm = np.stack([xw_ro, xw_ro_b]).transpose(0, 2, 1)
    dtwm = np.stack([f("dtproj_w"), f("dtproj_w_b")]).transpose(0, 2, 1)
    dtbm = np.stack([f("dtproj_b"), f("dtproj_b_b")])[..., None]
    shared = {
        "w_in": bf(f("in_proj_w").T),
        "convd": bf(convd),
        "convb": np.ascontiguousarray(
            cb.reshape(2, 2, 128, 1).transpose(2, 0, 1, 3)),
        "xw": bf(xwm.reshape(2, 2, 128, 40).transpose(2, 0, 1, 3)),
        "dtw": bf(dtwm.transpose(1, 0, 2)),                         # [8,2,256]
        "dtb": np.ascontiguousarray(
            dtbm.reshape(2, 2, 128, 1).transpose(2, 0, 1, 3)),
        "dpc": np.ascontiguousarray(
            np.stack([f("Dp"), f("Dp_b")])[..., None]
            .reshape(2, 2, 128, 1).transpose(2, 0, 1, 3)),
        "wout": bf(f("out_proj_w").T.reshape(2, 128, 128).transpose(1, 0, 2)),
        "ln1g": f("ln1_g").reshape(C, 1),
        "ln1b": f("ln1_b").reshape(C, 1),
        "ln2g": f("ln2_g").reshape(C, 1),
        "ln2b": f("ln2_b").reshape(C, 1),
    }
    return xin, shared, a_pow


def _unshard(core_outs):
    y = np.stack(core_outs)                       # [8, C, BC, T]
    y = y.transpose(0, 2, 3, 1).reshape(BSEQ, T, C)[:B * N]
    return np.ascontiguousarray(
        y.reshape(B, N, T, C).transpose(0, 2, 1, 3))


_CACHE = {}


def kernel(_trace=False, **inputs):
    xin, shared, a_pow = _prep(inputs)
    if "prog" not in _CACHE:
        _CACHE["prog"] = build_program(a_pow)
    nc = _CACHE["prog"]
    in_maps = [dict(shared, xin=xin[i]) for i in range(NCORES)]
    res = run_bass_kernel_spmd(nc, in_maps, core_ids=list(range(NCORES)),
                               trace=_trace)
    out = _unshard([r["out"] for r in res.results])
    if _trace:
        kernel.last_results = res
    return out
